# revision 2
# baseline (speedup 1.0000x reference)
"""Multi-head causal self-attention (B=4, S=2048, D=1024, H=16) on 8 trn2 cores.

Sharding: core c = (batch b = c//2, head-group g = c%2 of 8 heads).
Each core computes Q/K/V projections for its 8 heads over its batch's 2048
tokens, causal attention, and a PARTIAL output projection over its 512
feature dims. The host adds the two partial [2048, 1024] outputs per batch.
No on-device collectives.

v2 redesign vs v1 (570us baseline):
  - No causal-mask identity matmuls. Diagonal score tiles are computed
    trimmed to their valid q-range [128r, 512); exp is applied trimmed
    per-u; the within-tile upper triangle of the exp output is zeroed by a
    gpsimd affine_select; PV matmuls read only the valid (trimmed) range.
  - bf16 everywhere (x and weights converted on host): same PE rate as
    f32r but no >=256 free-dim constraint (trimmed diag matmuls cheap),
    half the DMA bytes, half the SBUF.
  - Single software-pipelined instruction stream: QKV projection chains
    and W_O chains are woven as PE filler between attention score/PV
    groups so the PE never idles while ACT (exp softmax, ~150us total,
    the phase-2 bottleneck engine) catches up. Weave quotas reserve the
    W_O chains of early q-slices for the late (ACT-heaviest) windows.
    PV matmuls for group g are emitted after the scores of group g+1 to
    hide exp latency; fillers go before scores so ring-reuse waits are
    absorbed by independent work.

On-core dataflow:
  QT[e,t], KT[e,t] = W @ x^T          (e on partitions -> head-dim-major)
  V'[t, h, 0:64] = x @ Wv^T, col 64 = 1.0   (ones column makes PV matmul
                                             also produce the softmax sum Z)
  St[s,q] = K^T-slices .T @ Q^T-slices      (scores transposed; pairs of
                                             64-row heads via PE quadrants)
  E = exp(St/8) via ACT from PSUM (trimmed to valid causal range)
  numerT[dv,q], Z[q] = V'^T.T @ E            (accumulated over s-tiles)
  attnT = numerT * (1/Z broadcast)
  out[t,e] = attnT-slices .T @ Wo^T-slices   (partial over this core's dims)
"""

from collections import deque

import numpy as np

import concourse.bass as bass
import concourse.mybir as mybir
import concourse.tile as tile
from concourse import bacc
from concourse.bass_utils import run_bass_kernel_spmd

F32 = mybir.dt.float32
BF16 = mybir.dt.bfloat16
MM_DT = BF16
AF = mybir.ActivationFunctionType

B = 4
S = 2048
D_MODEL = 1024
E = 512          # feature dims per core (8 heads x 64)
HEADS = 8        # heads per core
DK = 64
NQ = 4           # 512-token q-slices
ND = 8           # 128-dim d_model tiles
NT = 16          # 128-token tiles
SCALE = 0.125    # 1/sqrt(dk)

TRI_ON_DVE = True  # zero diag triangles via DVE mask-mul (vs gpsimd select)


def _emit(tc, xT, wqT, wkT, wvT, woT, out):
    nc = tc.nc
    with (
        tc.tile_pool(name="singles", bufs=1) as singles,
        tc.tile_pool(name="xp", bufs=3) as xp,
        tc.tile_pool(name="expp", bufs=6) as expp,
        tc.tile_pool(name="attp", bufs=12) as attp,
        tc.tile_pool(name="nrmp", bufs=4) as nrmp,
        tc.tile_pool(name="pvcp", bufs=4) as pvcp,
        tc.tile_pool(name="otp", bufs=4) as otp,
        tc.tile_pool(name="gps", bufs=2, space="PSUM") as gps,
    ):
        qt = [singles.tile([128, S], BF16, name=f"qt{i}") for i in range(4)]
        kt = [singles.tile([128, S], BF16, name=f"kt{i}") for i in range(4)]
        vt = singles.tile([128, NT, HEADS, DK + 1], BF16, name="vt")
        wot = singles.tile([128, 4, D_MODEL], BF16, name="wot")
        wq = singles.tile([128, ND, E], MM_DT, name="wq")
        wk = singles.tile([128, ND, E], MM_DT, name="wk")
        wv = singles.tile([128, ND, E], MM_DT, name="wv")

        # ---- initial DMAs: wq + x0 first (gate the first matmul chain) ----
        xt = {}
        xt[0] = xp.tile([128, ND, 512], MM_DT, tag="x", name="x0")
        nc.sync.dma_start(out=wq[:, 0:4, :], in_=wqT[:, 0:4, :])
        nc.scalar.dma_start(out=wq[:, 4:8, :], in_=wqT[:, 4:8, :])
        nc.gpsimd.dma_start(out=xt[0][:, 0:4, :], in_=xT[0, :, 0:4, :])
        nc.sync.dma_start(out=xt[0][:, 4:8, :], in_=xT[0, :, 4:8, :])
        nc.scalar.dma_start(out=wk[:, 0:4, :], in_=wkT[:, 0:4, :])
        nc.gpsimd.dma_start(out=wk[:, 4:8, :], in_=wkT[:, 4:8, :])
        nc.sync.dma_start(out=wv[:, 0:4, :], in_=wvT[:, 0:4, :])
        nc.scalar.dma_start(out=wv[:, 4:8, :], in_=wvT[:, 4:8, :])
        nc.gpsimd.dma_start(out=wot, in_=woT)
        xt[1] = xp.tile([128, ND, 512], MM_DT, tag="x", name="x1")
        nc.sync.dma_start(out=xt[1][:, 0:4, :], in_=xT[1, :, 0:4, :])
        nc.gpsimd.dma_start(out=xt[1][:, 4:8, :], in_=xT[1, :, 4:8, :])

        # ones column of V' (makes PV also produce the softmax denominator)
        tri = singles.tile([128, 128], BF16, name="tri") if TRI_ON_DVE else None
        with tc.tile_pool(name="scratch", bufs=1) as scratch:
            sc = scratch.tile([128, 128], F32, name="sc")
            nc.vector.memset(sc, 1.0)
            nc.vector.tensor_copy(vt[:, :, :, DK:DK + 1],
                                  sc.rearrange("p (a b) -> p a b", a=NT))
            if TRI_ON_DVE:
                # tri[p, c] = 1 where c >= p else 0 (keep-mask for diagonals)
                nc.gpsimd.affine_select(
                    out=sc, in_=sc, pattern=[[1, 128]],
                    compare_op=mybir.AluOpType.is_ge,
                    fill=0.0, base=0, channel_multiplier=-1)
                nc.vector.tensor_copy(tri, sc)

        # ---------------- filler chain machinery ----------------
        # Each chain emits one PSUM-accumulation chain (8 matmuls for QKV,
        # 4 for WO) + its copy-out. Chains are woven between attention
        # groups (filler first, so ring-reuse waits overlap independent
        # work). Weave quotas reserve early-slice WO chains for the late,
        # ACT-heaviest attention windows.
        emitted = set()
        registry = {}

        def q_chain(j, e):
            def go():
                tsl = slice(j * 512, (j + 1) * 512)
                esl = slice(e * 128, (e + 1) * 128)
                ps = gps.tile([128, 512], F32, tag="mm", name=f"q{j}_{e}")
                for d in range(ND):
                    nc.tensor.matmul(ps, wq[:, d, esl], xt[j][:, d, :],
                                     start=(d == 0), stop=(d == ND - 1))
                nc.vector.tensor_copy(qt[e][:, tsl], ps)
            return (f"q{j}_{e}", go)

        def k_chain(j, e):
            def go():
                tsl = slice(j * 512, (j + 1) * 512)
                esl = slice(e * 128, (e + 1) * 128)
                ps = gps.tile([128, 512], F32, tag="mm", name=f"k{j}_{e}")
                for d in range(ND):
                    nc.tensor.matmul(ps, wk[:, d, esl], xt[j][:, d, :],
                                     start=(d == 0), stop=(d == ND - 1))
                nc.vector.tensor_copy(kt[e][:, tsl], ps)
            return (f"k{j}_{e}", go)

        def v_chain(j, u):
            def go():
                i = 4 * j + u
                usl = slice(u * 128, (u + 1) * 128)
                ps = gps.tile([128, 512], F32, tag="mm", name=f"v{j}_{u}")
                for d in range(ND):
                    nc.tensor.matmul(ps, xt[j][:, d, usl], wv[:, d, :],
                                     start=(d == 0), stop=(d == ND - 1))
                nc.vector.tensor_copy(
                    vt[:, i, :, 0:DK],
                    ps.rearrange("p (h k) -> p h k", h=HEADS))
            return (f"v{j}_{u}", go)

        att_tiles = {}

        wo_pool = {j: (gps, "mm") for j in range(3)}  # j=3 set to tailp below

        def wo_chain(j, tt, eo):
            def go():
                ttsl = slice(tt * 128, (tt + 1) * 128)
                pool, tag = wo_pool[j]
                pso = pool.tile([128, 512], F32, tag=tag, name=f"o{j}{tt}{eo}")
                for f in range(4):
                    nc.tensor.matmul(pso, att_tiles[j][f][:, ttsl],
                                     wot[:, f, eo * 512:(eo + 1) * 512],
                                     start=(f == 0), stop=(f == 3))
                ot = otp.tile([128, 512], F32, tag="ot", name=f"ot{j}{tt}{eo}")
                nc.vector.tensor_copy(ot, pso)
                t0 = j * 512 + tt * 128
                nc.sync.dma_start(
                    out=out[t0:t0 + 128, eo * 512:(eo + 1) * 512], in_=ot)
            return (f"o{j}_{tt}_{eo}", go)

        def emit_named(name):
            if name not in emitted:
                registry[name]()
                emitted.add(name)

        def qkv_names(j):
            # order: per-hp Q,K first (deadline order), V early for PV
            names = []
            for e in range(2):
                names += [f"q{j}_{e}", f"k{j}_{e}", f"v{j}_{2 * e}",
                          f"v{j}_{2 * e + 1}"]
            for e in range(2, 4):
                names += [f"q{j}_{e}", f"k{j}_{e}"]
            return names

        def wo_names(j):
            return [f"o{j}_{tt}_{eo}" for tt in range(4) for eo in range(2)]

        for j in range(NQ):
            for e in range(4):
                for nm, go in (q_chain(j, e), k_chain(j, e)):
                    registry[nm] = go
            for u in range(4):
                nm, go = v_chain(j, u)
                registry[nm] = go

        # ---------------- attention, pipelined over q-slices ----------------
        from contextlib import ExitStack
        att_stack = ExitStack()
        stp = att_stack.enter_context(
            tc.tile_pool(name="stp", bufs=2, space="PSUM"))
        pvp = att_stack.enter_context(
            tc.tile_pool(name="pvp", bufs=2, space="PSUM"))
        for j in range(NQ):
            # prefetch x for slice j+2 (j, j+1 already in flight)
            if j + 2 < NQ:
                jx = j + 2
                xt[jx] = xp.tile([128, ND, 512], MM_DT, tag="x", name=f"x{jx}")
                nc.sync.dma_start(out=xt[jx][:, 0:4, :], in_=xT[jx, :, 0:4, :])
                nc.gpsimd.dma_start(out=xt[jx][:, 4:8, :], in_=xT[jx, :, 4:8, :])

            att_tiles[j] = [attp.tile([128, 512], BF16, tag="at",
                                      name=f"at{j}_{f}") for f in range(4)]

            # weave list for this window: next slice's QKV + reserved WO
            weave = deque()
            if j == 0:
                weave.extend(qkv_names(0))  # pulled early by deadline flushes
            if j + 1 < NQ:
                weave.extend(qkv_names(j + 1))
            if j == 2:
                weave.extend(wo_names(0))
            if j == 3:
                weave.extend(wo_names(1))
                weave.extend(wo_names(2))
            n_groups = 4 * 2 * (j + 1)
            quota_acc = 0.0
            quota_per_group = len(weave) / n_groups

            nst = 4 * (j + 1)
            pend = None  # delayed PV+norm closure (hides exp latency)
            for hp in range(4):
                hA, hB = 2 * hp, 2 * hp + 1
                for nm in (f"q{j}_{hp}", f"k{j}_{hp}"):
                    while nm not in emitted:
                        emit_named(weave.popleft() if weave else nm)
                pvA = pvp.tile([DK + 1, 512], F32, tag="pv", name=f"pvA{j}{hp}")
                pvB = pvp.tile([DK + 1, 512], F32, tag="pv", name=f"pvB{j}{hp}")
                for g in range(nst // 2):
                    # weave fillers first: independent work absorbs the
                    # st-ring / exp-latency waits of this group's scores.
                    # hp boundaries (g==0) get an extra chain: the ACT queue
                    # has a latency spike there (diag exp backlog + norm).
                    quota_acc += quota_per_group
                    while quota_acc >= 1.0 and weave:
                        emit_named(weave.popleft())
                        quota_acc -= 1.0

                    stA = stp.tile([128, 2, 512], F32, tag="st",
                                   name=f"stA{j}{hp}{g}")
                    stB = stp.tile([128, 2, 512], F32, tag="st",
                                   name=f"stB{j}{hp}{g}")
                    exA = expp.tile([128, 2, 512], BF16, tag="ex",
                                    name=f"exA{j}{hp}{g}")
                    exB = expp.tile([128, 2, 512], BF16, tag="ex",
                                    name=f"exB{j}{hp}{g}")
                    diag = g >= 2 * j  # last two groups of each j: diagonal
                    for u in range(2):
                        i = 2 * g + u
                        r = i - 4 * j
                        q0 = 128 * r if r >= 0 else 0
                        ssl = slice(i * 128, (i + 1) * 128)
                        qv = slice(j * 512 + q0, (j + 1) * 512)
                        nc.tensor.matmul(stA[:, u, q0:], kt[hp][0:64, ssl],
                                         qt[hp][0:64, qv], start=True,
                                         stop=True, tile_position=(0, 0))
                        nc.tensor.matmul(stB[:, u, q0:], kt[hp][64:128, ssl],
                                         qt[hp][64:128, qv], start=True,
                                         stop=True, tile_position=(64, 0))
                    # exp on ACT; diag groups trimmed per-u + triangle zero
                    if diag:
                        for ex, st in ((exA, stA), (exB, stB)):
                            for u in range(2):
                                r = 2 * g + u - 4 * j
                                q0 = 128 * r
                                nc.scalar.activation(ex[:, u, q0:],
                                                     st[:, u, q0:],
                                                     AF.Exp, scale=SCALE)
                                # zero upper triangle (keep where col >= p)
                                if TRI_ON_DVE:
                                    nc.vector.tensor_mul(
                                        ex[:, u, q0:q0 + 128],
                                        ex[:, u, q0:q0 + 128], tri)
                                else:
                                    nc.gpsimd.affine_select(
                                        out=ex[:, u, q0:q0 + 128],
                                        in_=ex[:, u, q0:q0 + 128],
                                        pattern=[[1, 128]],
                                        compare_op=mybir.AluOpType.is_ge,
                                        fill=0.0, base=0,
                                        channel_multiplier=-1)
                    else:
                        nc.scalar.activation(exA, stA, AF.Exp, scale=SCALE)
                        nc.scalar.activation(exB, stB, AF.Exp, scale=SCALE)

                    if pend is not None:
                        pend()

                    def make_pv(j=j, hp=hp, g=g, hA=hA, hB=hB, pvA=pvA,
                                pvB=pvB, exA=exA, exB=exB, nst=nst,
                                last=(g == nst // 2 - 1)):
                        def go():
                            for u in range(2):
                                i = 2 * g + u
                                r = i - 4 * j
                                q0 = 128 * r if r >= 0 else 0
                                nm = f"v{i // 4}_{i % 4}"
                                while nm not in emitted:
                                    emit_named(
                                        weave.popleft() if weave else nm)
                                nc.tensor.matmul(pvA[:, q0:], vt[:, i, hA, :],
                                                 exA[:, u, q0:],
                                                 start=(i == 0),
                                                 stop=(i == nst - 1))
                                nc.tensor.matmul(pvB[:, q0:], vt[:, i, hB, :],
                                                 exB[:, u, q0:],
                                                 start=(i == 0),
                                                 stop=(i == nst - 1))
                            if last:
                                # copy accumulators to SBUF (frees the pv
                                # PSUM banks early), then att = numer * 1/Z
                                for pvx, p0 in ((pvA, 0), (pvB, 64)):
                                    pvc = pvcp.tile([DK + 1, 512], F32,
                                                    tag="pvc",
                                                    name=f"pvc{j}{hp}{p0}")
                                    nc.vector.tensor_copy(pvc, pvx)
                                    rz = nrmp.tile([1, 512], F32, tag="rz",
                                                   name=f"rz{j}{hp}{p0}")
                                    bz = nrmp.tile([64, 512], F32, tag="bz",
                                                   name=f"bz{j}{hp}{p0}")
                                    nc.vector.reciprocal(rz, pvc[DK:DK + 1, :])
                                    nc.gpsimd.partition_broadcast(bz, rz)
                                    nc.vector.tensor_mul(
                                        att_tiles[j][hp][p0:p0 + 64, :],
                                        pvc[0:DK, :], bz)
                        return go
                    pend = make_pv()
            if pend is not None:
                pend()
                pend = None
            # drain this window's weave list; register this slice's WO chains
            while weave:
                emit_named(weave.popleft())
            for tt in range(4):
                for eo in range(2):
                    nm, go = wo_chain(j, tt, eo)
                    registry[nm] = go

        # tail: last slice's WO chains get a wider PSUM ring (st/pv banks
        # freed) so their f<3 matmuls pre-run before the final norms land
        att_stack.close()
        with tc.tile_pool(name="tailp", bufs=4, space="PSUM") as tailp:
            wo_pool[3] = (tailp, "tail")
            for nm in wo_names(3):
                emit_named(nm)


def build_nc(reps=1):
    nc = bacc.Bacc(None, target_bir_lowering=False, debug=False)
    xT = nc.dram_tensor("xT", [NQ, 128, ND, 512], MM_DT, kind="ExternalInput")
    wqT = nc.dram_tensor("wqT", [128, ND, E], MM_DT, kind="ExternalInput")
    wkT = nc.dram_tensor("wkT", [128, ND, E], MM_DT, kind="ExternalInput")
    wvT = nc.dram_tensor("wvT", [128, ND, E], MM_DT, kind="ExternalInput")
    woT = nc.dram_tensor("woT", [128, 4, D_MODEL], BF16, kind="ExternalInput")
    out = nc.dram_tensor("out", [S, D_MODEL], F32, kind="ExternalOutput")
    aps = (xT.ap(), wqT.ap(), wkT.ap(), wvT.ap(), woT.ap(), out.ap())
    with tile.TileContext(nc) as tc:
        if reps == 1:
            _emit(tc, *aps)
        else:
            with tc.For_i(0, reps, 1):
                _emit(tc, *aps)
    nc.compile()
    return nc


def make_in_maps(x, W_Q, W_K, W_V, W_O):
    import ml_dtypes
    bf = ml_dtypes.bfloat16
    in_maps = []
    for c in range(8):
        b, g = divmod(c, 2)
        sl = slice(g * E, (g + 1) * E)
        xt = x[b].T  # [D, S]
        # [j, dpart(128), dtile(8), t(512)] — partition-major so the bulk
        # DMA's flat element order matches the SBUF tile's [p, d, c] order
        xt4 = np.ascontiguousarray(
            xt.reshape(8, 128, 4, 512).transpose(2, 1, 0, 3)).astype(bf)
        def wtile(w):  # [D, E] -> [128, 8, E]
            return np.ascontiguousarray(
                w.reshape(8, 128, -1).transpose(1, 0, 2)).astype(bf)
        in_maps.append({
            "xT": xt4,
            "wqT": wtile(W_Q[sl, :].T),
            "wkT": wtile(W_K[sl, :].T),
            "wvT": wtile(W_V[sl, :].T),
            "woT": np.ascontiguousarray(
                W_O[:, sl].T.reshape(4, 128, 1024).transpose(1, 0, 2)
            ).astype(bf),
        })
    return in_maps


_NC_CACHE = None


def kernel(x, W_Q, W_K, W_V, W_O, _trace=False):
    global _NC_CACHE
    if _NC_CACHE is None:
        _NC_CACHE = build_nc()
    nc = _NC_CACHE
    in_maps = make_in_maps(x, W_Q, W_K, W_V, W_O)
    r = run_bass_kernel_spmd(nc, in_maps, list(range(8)), trace=_trace)
    kernel.last_result = r
    out = np.empty((B, S, D_MODEL), np.float32)
    for b in range(B):
        out[b] = r.results[2 * b]["out"] + r.results[2 * b + 1]["out"]
    return out
